# revision 18
# baseline (speedup 1.0000x reference)
"""Trainium2 Bass kernel for nn_DistanceDecoder (moe_routing).

reference:
    comp_b  = components[object_labels]            # [B, 32, 6144]
    mean_b  = means[object_labels]                 # [B, 6144]
    out     = einsum('bp,bpo->bo', lattent, comp_b) + mean_b

Strategy (8 NeuronCores):
  * Shard OUT_DIM (6144) 8-ways -> each core owns a 768-wide column slice
    and the full batch.  Per-core HBM traffic ~2.8 MB (fp16 table slice +
    fp16 output) vs 18 MB for the batch-parallel split.
  * Host stable-sorts the batch by label (MoE dispatch) and appends the
    per-object mean as a 33rd row of each object's [32, 768] component
    block with a matching constant-1.0 row in the latent matrix, so
    gather + vecmat + mean-add is a single block-banded matmul
        out_T[768, 1024] = C2aug^T @ Epack
    over 7 K-tiles of 3 objects (K = 3*33 = 99 rows).  Sorted-batch
    column ranges per K-tile are baked into the instruction stream.
  * All operands fp16 (same error class as fp32r PE rounding); the
    OUTPUT is also stored fp16 (halves the dominant output transfer;
    rel-err ~8e-4 vs the 2e-2 gate) and cast back to f32 on the host.
  * DMA issue is the hidden serial cost (~0.6-1.0us of issuing-engine
    time per dma_start, regardless of size) -> spread issues across the
    three DMA-capable engines: sync (epack halves + out 0/2/4), scalar
    (comp_a, comp_c + out 5b), gpsimd SWDGE (comp_b + out 1/3 + out 5a).
    Inputs are issued before any output on each engine.
  * PSUM drain split across DVE (seg 0) and Activation (seg 1) per
    chunk; the final chunk's two 512-col halves go out as two parallel
    half-DMAs so the tail is one half-transfer deep.
  * Host applies the inverse permutation + f32 cast at the end.

Measured fixed NEFF overhead (startup + exit barrier + semaphore-sweep
teardown) is ~10.8us on this stack; a trivial 1-DMA kernel measures
14.1us.  Optimization therefore targets the ~16us body: parallel issue,
fp16 output, and a short tail.
"""

import numpy as np

BATCH = 1024
PCA = 32
ROWS = PCA + 1             # 32 components + 1 mean row per object
OUT_DIM = 6144
NOBJ = 20
NCORES = 8
SLICE = OUT_DIM // NCORES  # 768
NCHUNK = SLICE // 128      # 6 chunks of 128 output rows (out_T partitions)
OBJ_PER_KT = 3             # objects per K-tile -> K = 3*33 = 99 <= 128
KTILES = (NOBJ + OBJ_PER_KT - 1) // OBJ_PER_KT  # 7
KP = OBJ_PER_KT * ROWS     # 99 partitions per K-tile
SEGS = [(0, 512), (512, 1024)]  # PSUM bank segments

_NC_CACHE: dict = {}


def _kheight(t: int) -> int:
    return (min(OBJ_PER_KT * (t + 1), NOBJ) - OBJ_PER_KT * t) * ROWS


def _build_nc(ranges: tuple):
    """Build + compile the single-core Bass program (SPMD across 8 cores).

    ranges: KTILES+1 ints; ranges[t]..ranges[t+1] is the sorted-batch column
    range whose labels fall in objects [3t, 3t+3) — baked into the
    instruction stream as matmul free-dim offsets.
    """
    import concourse.mybir as mybir
    from concourse import bacc
    from concourse.tile import TileContext

    f16 = mybir.dt.float16
    f32 = mybir.dt.float32

    nc = bacc.Bacc("TRN2", target_bir_lowering=False, debug=False)

    # cols of comp: j*(KTILES*128) + t*128 + m   (j = out_T row chunk)
    CCOLS = KTILES * 128
    comp_d = nc.dram_tensor("comp", [KP, NCHUNK * CCOLS], f16, kind="ExternalInput")
    epack_d = nc.dram_tensor("epack", [KP, BATCH], f16, kind="ExternalInput")
    out_d = nc.dram_tensor("out", [SLICE, BATCH], f16, kind="ExternalOutput")

    with TileContext(nc) as tc:
        with (
            tc.tile_pool(name="const", bufs=1) as cpool,
            tc.tile_pool(name="outp", bufs=6) as opool,
            tc.tile_pool(name="ps", bufs=8, space="PSUM") as pspool,
        ):
            # Input DMAs spread across the three DMA-capable engines.
            # A DMA's completion sem fires only when the WHOLE transfer
            # lands and all in-flight queues share ~360 GB/s, so the
            # first-matmul gate must be a SMALL dedicated pair: epack
            # halves (sync HWDGE) + chunk-0 comp (gpsimd SWDGE).  Later
            # chunks ride behind on the same queues, arriving just in
            # time.  scalar's first slot is burned by the hoisted
            # ACT_TABLE_LOAD (1.28us) so it gets the mid group only.
            # DMA engines round-robin packets across ACTIVE queues, so
            # co-scheduled transfers all complete near the end of the
            # combined stream.  Priority therefore comes from queue-level
            # ordering: chunk-k comp rides BEHIND chunk-(k-1) on the same
            # gpsimd SWDGE queue, and sync carries only the small epack
            # halves, so the chunk-0 gate (epack0+comp0, 277KB) clears
            # the moment those bytes land instead of after all 1.26MB.
            # There are only 3 DGE queues (sync=Q1, scalar=Q10,
            # gpsimd=Q0) and they fair-share ~360 GB/s aggregate
            # (~110 GB/s each while all three stream).  A queue's DMAs
            # complete strictly in order, so total 2.76MB of traffic is
            # BYTE-BALANCED across the queues with deadline ordering:
            # earliest-needed pieces first on each queue.
            epack = cpool.tile([KP, BATCH], f16)
            compt = [
                cpool.tile([KP, CCOLS], f16, name=f"c{j}") for j in range(NCHUNK)
            ]
            # comp0 rides sync's HWDGE right behind epack0 (SWDGE descs
            # materialize ~1.5us late, too slow for the first-matmul
            # gate); epack1 takes scalar's first slot.
            nc.sync.dma_start(out=epack[:, 0:512], in_=epack_d[:, 0:512])
            nc.scalar.dma_start(out=epack[:, 512:BATCH], in_=epack_d[:, 512:BATCH])
            nc.sync.dma_start(out=compt[0], in_=comp_d[:, 0:CCOLS])
            nc.scalar.dma_start(out=compt[1], in_=comp_d[:, CCOLS : 2 * CCOLS])
            nc.gpsimd.dma_start(out=compt[2], in_=comp_d[:, 2 * CCOLS : 3 * CCOLS])
            nc.sync.dma_start(out=compt[3], in_=comp_d[:, 3 * CCOLS : 4 * CCOLS])
            nc.gpsimd.dma_start(out=compt[4], in_=comp_d[:, 4 * CCOLS : 5 * CCOLS])
            nc.sync.dma_start(out=compt[5], in_=comp_d[:, 5 * CCOLS :])

            for j in range(NCHUNK):
                compj = compt[j]
                out_sb = opool.tile([128, BATCH], f16, tag="out_sb", name=f"osb{j}")
                for h, (lo_h, hi_h) in enumerate(SEGS):
                    ps = pspool.tile([128, 512], f32, tag="ps", name=f"ps{j}_{h}")
                    pieces = []
                    for t in range(KTILES):
                        lo = max(ranges[t], lo_h)
                        hi = min(ranges[t + 1], hi_h)
                        if lo < hi:
                            pieces.append((t, lo, hi))
                    # disjoint column pieces cover the bank; first starts the
                    # accumulation group, later ones land on untouched
                    # elements (per-element has_written => plain writes)
                    for i, (t, lo, hi) in enumerate(pieces):
                        kh = _kheight(t)
                        nc.tensor.matmul(
                            ps[:, lo - lo_h : hi - lo_h],
                            compj[:kh, t * 128 : (t + 1) * 128],
                            epack[:kh, lo:hi],
                            start=(i == 0),
                            stop=(i == len(pieces) - 1),
                        )
                    # drain split over both PSUM-capable engines (f32->f16)
                    if h == 0:
                        nc.vector.tensor_copy(out=out_sb[:, lo_h:hi_h], in_=ps)
                    else:
                        nc.scalar.copy(out_sb[:, lo_h:hi_h], ps)
                if j < NCHUNK - 1:
                    # byte-balance the output stream across all 3 queues;
                    # scalar gets only out2 so its ACTIVATE drain chain
                    # (which gates the final chunk) is never pushed out
                    eng = (nc.gpsimd, nc.sync, nc.scalar, nc.gpsimd, nc.scalar)[j]
                    eng.dma_start(out=out_d[j * 128 : (j + 1) * 128, :], in_=out_sb)
                else:
                    # final chunk: two parallel half-DMAs, each gated only on
                    # its own seg's drain -> tail is one half-transfer deep.
                    # sync (fast HWDGE, idle by now) takes seg 0; scalar
                    # issues seg 1 right after its own drain of it.
                    nc.sync.dma_start(
                        out=out_d[j * 128 : (j + 1) * 128, 0:512],
                        in_=out_sb[:, 0:512],
                    )
                    nc.scalar.dma_start(
                        out=out_d[j * 128 : (j + 1) * 128, 512:BATCH],
                        in_=out_sb[:, 512:BATCH],
                    )

    nc.compile()
    return nc


def _get_nc(ranges: tuple):
    if ranges not in _NC_CACHE:
        _NC_CACHE[ranges] = _build_nc(ranges)
    return _NC_CACHE[ranges]


def _prepare(lattent_codes, object_labels, means, components):
    x = np.ascontiguousarray(np.asarray(lattent_codes), dtype=np.float32)
    labels = np.asarray(object_labels).astype(np.int64)
    means = np.ascontiguousarray(np.asarray(means), dtype=np.float32)
    comp = np.ascontiguousarray(np.asarray(components), dtype=np.float32)

    perm = np.argsort(labels, kind="stable")
    ls = labels[perm]
    xs = x[perm]  # [B, 32]

    counts = np.bincount(ls, minlength=NOBJ)
    cum = np.concatenate([[0], np.cumsum(counts)])
    ranges = tuple(
        int(cum[min(OBJ_PER_KT * t, NOBJ)]) for t in range(KTILES + 1)
    )

    # Epack[(l%3)*33 + p, i] = xs[i, p]; row (l%3)*33+32 = 1.0
    band = (ls % OBJ_PER_KT).astype(np.int64)
    epack = np.zeros((KP, BATCH), np.float16)
    rows = band[None, :] * ROWS + np.arange(PCA)[:, None]  # [32, B]
    epack[rows, np.arange(BATCH)[None, :]] = xs.T.astype(np.float16)
    epack[band * ROWS + PCA, np.arange(BATCH)] = 1.0

    # augmented component table: per object 32 component rows + 1 mean row
    m2 = np.concatenate([comp, means[:, None, :]], axis=1)  # [20, 33, OUT]
    m2 = m2.reshape(NOBJ * ROWS, OUT_DIM)

    in_maps = []
    CCOLS = KTILES * 128
    for c in range(NCORES):
        sl = slice(c * SLICE, (c + 1) * SLICE)
        arr = np.zeros((KP, NCHUNK, KTILES, 128), np.float16)
        for t in range(KTILES):
            kh = _kheight(t)
            blk = m2[KP * t : KP * t + kh, sl]  # [kh, 768]
            arr[:kh, :, t, :] = blk.reshape(kh, NCHUNK, 128).astype(np.float16)
        comp_host = np.ascontiguousarray(arr.reshape(KP, NCHUNK * CCOLS))
        in_maps.append({"comp": comp_host, "epack": epack})
    return in_maps, ranges, perm


def _assemble(results, perm):
    out_s = np.empty((BATCH, OUT_DIM), np.float32)
    for c in range(NCORES):
        out_s[:, c * SLICE : (c + 1) * SLICE] = (
            results[c]["out"].astype(np.float32).T
        )
    out = np.empty_like(out_s)
    out[perm] = out_s
    return out


def run(inputs: dict, trace: bool = False):
    """Run on hardware; returns (full output, BassKernelResults)."""
    from concourse.bass_utils import run_bass_kernel_spmd

    in_maps, ranges, perm = _prepare(**inputs)
    nc = _get_nc(ranges)
    res = run_bass_kernel_spmd(
        nc, in_maps, core_ids=list(range(NCORES)), trace=trace
    )
    return _assemble(res.results, perm), res


def kernel(lattent_codes, object_labels, means, components) -> np.ndarray:
    out, _ = run(
        {
            "lattent_codes": lattent_codes,
            "object_labels": object_labels,
            "means": means,
            "components": components,
        }
    )
    return out


# revision 20
# speedup vs baseline: 1.0600x; 1.0600x over previous
"""Trainium2 Bass kernel for nn_DistanceDecoder (moe_routing).

reference:
    comp_b  = components[object_labels]            # [B, 32, 6144]
    mean_b  = means[object_labels]                 # [B, 6144]
    out     = einsum('bp,bpo->bo', lattent, comp_b) + mean_b

Strategy (8 NeuronCores):
  * Shard OUT_DIM (6144) 8-ways -> each core owns a 768-wide column slice
    and the full batch.  Per-core HBM traffic ~2.8 MB (fp16 table slice +
    fp16 output) vs 18 MB for the batch-parallel split.
  * Host stable-sorts the batch by label (MoE dispatch) and appends the
    per-object mean as a 33rd row of each object's [32, 768] component
    block with a matching constant-1.0 row in the latent matrix, so
    gather + vecmat + mean-add is a single block-banded matmul
        out_T[768, 1024] = C2aug^T @ Epack
    over 7 K-tiles of 3 objects (K = 3*33 = 99 rows).  Sorted-batch
    column ranges per K-tile are baked into the instruction stream.
  * All operands fp16 (same error class as fp32r PE rounding); the
    OUTPUT is also stored fp16 (halves the dominant output transfer;
    rel-err ~8e-4 vs the 2e-2 gate) and cast back to f32 on the host.
  * DMA issue is the hidden serial cost (~0.6-1.0us of issuing-engine
    time per dma_start, regardless of size) -> spread issues across the
    three DMA-capable engines: sync (epack halves + out 0/2/4), scalar
    (comp_a, comp_c + out 5b), gpsimd SWDGE (comp_b + out 1/3 + out 5a).
    Inputs are issued before any output on each engine.
  * PSUM drain split across DVE (seg 0) and Activation (seg 1) per
    chunk; the final chunk's two 512-col halves go out as two parallel
    half-DMAs so the tail is one half-transfer deep.
  * Host applies the inverse permutation + f32 cast at the end.

Measured fixed NEFF overhead (startup + exit barrier + semaphore-sweep
teardown) is ~10.8us on this stack; a trivial 1-DMA kernel measures
14.1us.  Optimization therefore targets the ~16us body: parallel issue,
fp16 output, and a short tail.
"""

import numpy as np

BATCH = 1024
PCA = 32
ROWS = PCA + 1             # 32 components + 1 mean row per object
OUT_DIM = 6144
NOBJ = 20
NCORES = 8
SLICE = OUT_DIM // NCORES  # 768
NCHUNK = SLICE // 128      # 6 chunks of 128 output rows (out_T partitions)
OBJ_PER_KT = 3             # objects per K-tile -> K = 3*33 = 99 <= 128
KTILES = (NOBJ + OBJ_PER_KT - 1) // OBJ_PER_KT  # 7
KP = OBJ_PER_KT * ROWS     # 99 partitions per K-tile
SEGS = [(0, 512), (512, 1024)]  # PSUM bank segments

_NC_CACHE: dict = {}


def _kheight(t: int) -> int:
    return (min(OBJ_PER_KT * (t + 1), NOBJ) - OBJ_PER_KT * t) * ROWS


def _build_nc(ranges: tuple):
    """Build + compile the single-core Bass program (SPMD across 8 cores).

    ranges: KTILES+1 ints; ranges[t]..ranges[t+1] is the sorted-batch column
    range whose labels fall in objects [3t, 3t+3) — baked into the
    instruction stream as matmul free-dim offsets.
    """
    import concourse.mybir as mybir
    from concourse import bacc
    from concourse.tile import TileContext

    f16 = mybir.dt.float16
    f32 = mybir.dt.float32

    nc = bacc.Bacc("TRN2", target_bir_lowering=False, debug=False)

    # cols of comp: j*(KTILES*128) + t*128 + m   (j = out_T row chunk)
    CCOLS = KTILES * 128
    comp_d = nc.dram_tensor("comp", [KP, NCHUNK * CCOLS], f16, kind="ExternalInput")
    epack_d = nc.dram_tensor("epack", [KP, BATCH], f16, kind="ExternalInput")
    out_d = nc.dram_tensor("out", [SLICE, BATCH], f16, kind="ExternalOutput")

    with TileContext(nc) as tc:
        with (
            tc.tile_pool(name="const", bufs=1) as cpool,
            tc.tile_pool(name="outp", bufs=6) as opool,
            tc.tile_pool(name="ps", bufs=8, space="PSUM") as pspool,
        ):
            # Input DMAs spread across the three DMA-capable engines.
            # A DMA's completion sem fires only when the WHOLE transfer
            # lands and all in-flight queues share ~360 GB/s, so the
            # first-matmul gate must be a SMALL dedicated pair: epack
            # halves (sync HWDGE) + chunk-0 comp (gpsimd SWDGE).  Later
            # chunks ride behind on the same queues, arriving just in
            # time.  scalar's first slot is burned by the hoisted
            # ACT_TABLE_LOAD (1.28us) so it gets the mid group only.
            # DMA engines round-robin packets across ACTIVE queues, so
            # co-scheduled transfers all complete near the end of the
            # combined stream.  Priority therefore comes from queue-level
            # ordering: chunk-k comp rides BEHIND chunk-(k-1) on the same
            # gpsimd SWDGE queue, and sync carries only the small epack
            # halves, so the chunk-0 gate (epack0+comp0, 277KB) clears
            # the moment those bytes land instead of after all 1.26MB.
            # There are only 3 DGE queues (sync=Q1, scalar=Q10,
            # gpsimd=Q0) and they fair-share ~360 GB/s aggregate
            # (~110 GB/s each while all three stream).  A queue's DMAs
            # complete strictly in order, so total 2.76MB of traffic is
            # BYTE-BALANCED across the queues with deadline ordering:
            # earliest-needed pieces first on each queue.
            epack = cpool.tile([KP, BATCH], f16)
            compt = [
                cpool.tile([KP, CCOLS], f16, name=f"c{j}") for j in range(NCHUNK)
            ]
            nc.sync.dma_start(out=epack[:, 0:512], in_=epack_d[:, 0:512])
            nc.gpsimd.dma_start(out=compt[0], in_=comp_d[:, 0:CCOLS])
            nc.sync.dma_start(out=epack[:, 512:BATCH], in_=epack_d[:, 512:BATCH])
            nc.scalar.dma_start(out=compt[1], in_=comp_d[:, CCOLS : 2 * CCOLS])
            nc.gpsimd.dma_start(out=compt[2], in_=comp_d[:, 2 * CCOLS : 3 * CCOLS])
            nc.sync.dma_start(out=compt[3], in_=comp_d[:, 3 * CCOLS : 4 * CCOLS])
            nc.gpsimd.dma_start(out=compt[4], in_=comp_d[:, 4 * CCOLS : 5 * CCOLS])
            nc.sync.dma_start(out=compt[5], in_=comp_d[:, 5 * CCOLS :])

            for j in range(NCHUNK):
                compj = compt[j]
                out_sb = opool.tile([128, BATCH], f16, tag="out_sb", name=f"osb{j}")
                for h, (lo_h, hi_h) in enumerate(SEGS):
                    ps = pspool.tile([128, 512], f32, tag="ps", name=f"ps{j}_{h}")
                    pieces = []
                    for t in range(KTILES):
                        lo = max(ranges[t], lo_h)
                        hi = min(ranges[t + 1], hi_h)
                        if lo < hi:
                            pieces.append((t, lo, hi))
                    # disjoint column pieces cover the bank; first starts the
                    # accumulation group, later ones land on untouched
                    # elements (per-element has_written => plain writes)
                    for i, (t, lo, hi) in enumerate(pieces):
                        kh = _kheight(t)
                        nc.tensor.matmul(
                            ps[:, lo - lo_h : hi - lo_h],
                            compj[:kh, t * 128 : (t + 1) * 128],
                            epack[:kh, lo:hi],
                            start=(i == 0),
                            stop=(i == len(pieces) - 1),
                        )
                    # drain split over both PSUM-capable engines (f32->f16)
                    if h == 0:
                        nc.vector.tensor_copy(out=out_sb[:, lo_h:hi_h], in_=ps)
                    else:
                        nc.scalar.copy(out_sb[:, lo_h:hi_h], ps)
                if j < NCHUNK - 1:
                    # byte-balance the output stream across all 3 queues;
                    # scalar gets only out2 so its ACTIVATE drain chain
                    # (which gates the final chunk) is never pushed out
                    eng = (nc.gpsimd, nc.sync, nc.scalar, nc.gpsimd, nc.sync)[j]
                    eng.dma_start(out=out_d[j * 128 : (j + 1) * 128, :], in_=out_sb)
                else:
                    # final chunk: two parallel half-DMAs, each gated only on
                    # its own seg's drain -> tail is one half-transfer deep.
                    # sync (fast HWDGE, idle by now) takes seg 0; scalar
                    # issues seg 1 right after its own drain of it.
                    nc.sync.dma_start(
                        out=out_d[j * 128 : (j + 1) * 128, 0:512],
                        in_=out_sb[:, 0:512],
                    )
                    nc.scalar.dma_start(
                        out=out_d[j * 128 : (j + 1) * 128, 512:BATCH],
                        in_=out_sb[:, 512:BATCH],
                    )

    nc.compile()
    return nc


def _get_nc(ranges: tuple):
    if ranges not in _NC_CACHE:
        _NC_CACHE[ranges] = _build_nc(ranges)
    return _NC_CACHE[ranges]


def _prepare(lattent_codes, object_labels, means, components):
    x = np.ascontiguousarray(np.asarray(lattent_codes), dtype=np.float32)
    labels = np.asarray(object_labels).astype(np.int64)
    means = np.ascontiguousarray(np.asarray(means), dtype=np.float32)
    comp = np.ascontiguousarray(np.asarray(components), dtype=np.float32)

    perm = np.argsort(labels, kind="stable")
    ls = labels[perm]
    xs = x[perm]  # [B, 32]

    counts = np.bincount(ls, minlength=NOBJ)
    cum = np.concatenate([[0], np.cumsum(counts)])
    ranges = tuple(
        int(cum[min(OBJ_PER_KT * t, NOBJ)]) for t in range(KTILES + 1)
    )

    # Epack[(l%3)*33 + p, i] = xs[i, p]; row (l%3)*33+32 = 1.0
    band = (ls % OBJ_PER_KT).astype(np.int64)
    epack = np.zeros((KP, BATCH), np.float16)
    rows = band[None, :] * ROWS + np.arange(PCA)[:, None]  # [32, B]
    epack[rows, np.arange(BATCH)[None, :]] = xs.T.astype(np.float16)
    epack[band * ROWS + PCA, np.arange(BATCH)] = 1.0

    # augmented component table: per object 32 component rows + 1 mean row
    m2 = np.concatenate([comp, means[:, None, :]], axis=1)  # [20, 33, OUT]
    m2 = m2.reshape(NOBJ * ROWS, OUT_DIM)

    in_maps = []
    CCOLS = KTILES * 128
    for c in range(NCORES):
        sl = slice(c * SLICE, (c + 1) * SLICE)
        arr = np.zeros((KP, NCHUNK, KTILES, 128), np.float16)
        for t in range(KTILES):
            kh = _kheight(t)
            blk = m2[KP * t : KP * t + kh, sl]  # [kh, 768]
            arr[:kh, :, t, :] = blk.reshape(kh, NCHUNK, 128).astype(np.float16)
        comp_host = np.ascontiguousarray(arr.reshape(KP, NCHUNK * CCOLS))
        in_maps.append({"comp": comp_host, "epack": epack})
    return in_maps, ranges, perm


def _assemble(results, perm):
    out_s = np.empty((BATCH, OUT_DIM), np.float32)
    for c in range(NCORES):
        out_s[:, c * SLICE : (c + 1) * SLICE] = (
            results[c]["out"].astype(np.float32).T
        )
    out = np.empty_like(out_s)
    out[perm] = out_s
    return out


def run(inputs: dict, trace: bool = False):
    """Run on hardware; returns (full output, BassKernelResults)."""
    from concourse.bass_utils import run_bass_kernel_spmd

    in_maps, ranges, perm = _prepare(**inputs)
    nc = _get_nc(ranges)
    res = run_bass_kernel_spmd(
        nc, in_maps, core_ids=list(range(NCORES)), trace=trace
    )
    return _assemble(res.results, perm), res


def kernel(lattent_codes, object_labels, means, components) -> np.ndarray:
    out, _ = run(
        {
            "lattent_codes": lattent_codes,
            "object_labels": object_labels,
            "means": means,
            "components": components,
        }
    )
    return out
